# revision 7
# baseline (speedup 1.0000x reference)
"""MoE layer (top-2 routing, 8 experts) on 8 Trainium2 NeuronCores.

Strategy (expert parallelism, per sharding hint):
  - Host computes the gate (logits -> top-k -> softmax) and routes tokens:
    expert e's tokens are gathered, padded to a common capacity C, and sent
    to core e (host-side all-to-all).
  - Core e runs the expert FFN for its tokens in a transpose-free layout:
        mm1:  h^T[f,c] = sum_k W1_blk[k,f].T @ x^T[k,c]
        gelu: ACT engine, exact (erf) Gelu, bias b1 fused
        mm2:  y^T[d,c] = sum_f W2_blk[f,d].T @ h^T[f,c]
    Activations/weights bf16 (full PE rate), accumulation fp32 in PSUM.
  - W1+W2 (16MB bf16) are loaded into SBUF ONCE and stay resident; x is
    preloaded in full.  After the initial ~20MB of DMA the PE never waits
    on HBM, which removes the per-chunk weight-reload stalls of the
    streaming version (88MB HBM traffic -> 21MB).
  - y is stored as bf16 (host combine is fp32) to halve store traffic.

Hardcoded problem shape: x [4, 2048, 1024], E=8 experts, D=1024, F=4096.
"""

import numpy as np
import ml_dtypes

import concourse.bass as bass
import concourse.mybir as mybir
import concourse.tile as tile
from concourse import bacc
from concourse.bass_utils import run_bass_kernel_spmd

D = 1024
F = 4096
E = 8
KD = D // 128   # 8 k-tiles over D
KF = F // 128   # 32 k-tiles over F
NT = 512        # max token chunk (PSUM bank = 512 fp32)

_KERNEL_CACHE = {}


def _chunks(C):
    """Token chunks: one narrow head chunk, then equal wide chunks (<= NT).

    The head chunk (~320) shrinks the x-prefetch the PE needs before its
    first matmul can retire (the whole chunk's x is consumed within the
    first f-tile).  All widths are >= ~264 so LDWEIGHTS (~107ns) stays
    hidden under the matmul streaming time, and multiples of 8.
    """
    head = 320
    if C <= head + 264:
        head = 0 if C <= NT else C // 2 & ~7
    rest = C - head
    nch = -(-rest // NT)
    base = (rest // nch) & ~7
    rem = rest - base * nch
    assert rem % 8 == 0
    widths = ([head] if head else []) + [
        base + 8 * (i < rem // 8) for i in range(nch)
    ]
    out, c0 = [], 0
    for w in widths:
        out.append((c0, w))
        c0 += w
    assert c0 == C
    return out


def _build_kernel(C: int):
    """Per-core expert-FFN kernel for capacity C (multiple of 64)."""
    assert C % 64 == 0
    bf16 = mybir.dt.bfloat16
    f32 = mybir.dt.float32

    nc = bacc.Bacc("TRN2", target_bir_lowering=False, debug=False, num_devices=8)

    # All operands packed host-side into partition-major [128, ...] layouts
    # so every load is one large fully-contiguous-per-partition DMA.
    xT = nc.dram_tensor("xT", [128, KD * C], bf16, kind="ExternalInput")
    w1 = nc.dram_tensor("w1", [128, KF * KD * 128], bf16, kind="ExternalInput")
    w2 = nc.dram_tensor("w2", [128, KD * KF * 128], bf16, kind="ExternalInput")
    b1 = nc.dram_tensor("b1", [128, KF], f32, kind="ExternalInput")
    b2 = nc.dram_tensor("b2", [128, KD], f32, kind="ExternalInput")
    yT = nc.dram_tensor("yT", [128, KD * C], bf16, kind="ExternalOutput")

    W1BLK = 4  # f-tiles per w1 load (1MB DMAs)

    with tile.TileContext(nc) as tc:
        with (
            tc.tile_pool(name="const", bufs=1) as const,
            tc.tile_pool(name="hp", bufs=1) as hp,
            tc.tile_pool(name="yp", bufs=4) as yp,
            tc.tile_pool(name="psA", bufs=4, space="PSUM") as psA,
            tc.tile_pool(name="psB", bufs=4, space="PSUM") as psB,
        ):
            chunks = _chunks(C)

            b1_sb = const.tile([128, KF], f32)
            nc.sync.dma_start(b1_sb[:], b1[:])
            b2_sb = const.tile([128, KD], f32)
            nc.sync.dma_start(b2_sb[:], b2[:])

            # Head choreography: the first chunk's x slices ride the Sync
            # (HWDGE) queue ahead of the weights so the PE's first f-tile
            # retires ~1us in; later chunks' x stream on the GpSimd queue
            # concurrently with the weight loads.
            x_sb = const.tile([128, KD * C], bf16)

            def load_x(chunk, queue):
                c0, w = chunk
                for k in range(KD):
                    queue.dma_start(
                        x_sb[:, k * C + c0 : k * C + c0 + w],
                        xT[:, k * C + c0 : k * C + c0 + w],
                    )

            # Chunk 0's x leads the Sync queue (PE's first dependency);
            # chunk 1's x rides the GpSimd queue concurrently.  Chunks 2+
            # aren't needed until ~100us, so they queue BEHIND the weights
            # instead of stealing HBM bandwidth from w1 in the critical
            # first ~25us.
            load_x(chunks[0], nc.sync)
            if len(chunks) > 1:
                load_x(chunks[1], nc.gpsimd)

            # w1: first 4 f-tiles individually (PE consumes an f-tile every
            # ~1.1us warm), then 1MB 4-f blocks; w2 per-d behind them.
            w1_sb = const.tile([128, KF * KD * 128], bf16)
            for f0 in range(W1BLK):
                lo, hi = f0 * KD * 128, (f0 + 1) * KD * 128
                nc.sync.dma_start(w1_sb[:, lo:hi], w1[:, lo:hi])
            for f0 in range(W1BLK, KF, W1BLK):
                lo, hi = f0 * KD * 128, (f0 + W1BLK) * KD * 128
                nc.sync.dma_start(w1_sb[:, lo:hi], w1[:, lo:hi])
            w2_sb = const.tile([128, KD * KF * 128], bf16)
            for d in range(KD):
                lo, hi = d * KF * 128, (d + 1) * KF * 128
                nc.sync.dma_start(w2_sb[:, lo:hi], w2[:, lo:hi])
            for chunk in chunks[2:]:
                load_x(chunk, nc.gpsimd)

            for c0, w in chunks:
                h_t = hp.tile([128, KF, NT], bf16)
                for f in range(KF):
                    ps = psA.tile([128, NT], f32)
                    for k in range(KD):
                        nc.tensor.matmul(
                            ps[:, :w],
                            w1_sb[:, (f * KD + k) * 128 : (f * KD + k + 1) * 128],
                            x_sb[:, k * C + c0 : k * C + c0 + w],
                            start=(k == 0),
                            stop=(k == KD - 1),
                        )
                    nc.scalar.activation(
                        h_t[:, f, :w],
                        ps[:, :w],
                        mybir.ActivationFunctionType.Gelu,
                        bias=b1_sb[:, f : f + 1],
                    )

                for d in range(KD):
                    ps2 = psB.tile([128, NT], f32)
                    for k2 in range(KF):
                        nc.tensor.matmul(
                            ps2[:, :w],
                            w2_sb[:, (d * KF + k2) * 128 : (d * KF + k2 + 1) * 128],
                            h_t[:, k2, :w],
                            start=(k2 == 0),
                            stop=(k2 == KF - 1),
                        )
                    y_t = yp.tile([128, NT], bf16)
                    nc.vector.tensor_scalar_add(y_t[:, :w], ps2[:, :w], b2_sb[:, d : d + 1])
                    # y stores on the Sync (HWDGE) queue: the end-of-kernel
                    # drain of the GpSimd SWDGE queue costs ~5us extra.
                    nc.sync.dma_start(yT[:, d * C + c0 : d * C + c0 + w], y_t[:, :w])

    nc.compile()
    return nc


def _get_kernel(C: int):
    if C not in _KERNEL_CACHE:
        _KERNEL_CACHE[C] = _build_kernel(C)
    return _KERNEL_CACHE[C]


def _route(xf, Wg, bg, top_k):
    """Replicate the reference gate: logits -> top-k -> softmax."""
    logits = xf.astype(np.float32) @ Wg.astype(np.float32) + bg.astype(np.float32)
    # jax.lax.top_k: values sorted descending, ties broken by lower index.
    order = np.argsort(-logits, axis=1, kind="stable")
    sel = order[:, :top_k]                                      # [T, K]
    vals = np.take_along_axis(logits, sel, axis=1)              # [T, K]
    vmax = vals.max(axis=1, keepdims=True)
    ex = np.exp((vals - vmax).astype(np.float32))
    w = ex / ex.sum(axis=1, keepdims=True)                      # [T, K]
    return sel, w.astype(np.float32)


def _plan(x, Wg, bg, top_k):
    """Routing plan: token indices + gate weight per expert, capacity C."""
    B, S, _ = x.shape
    xf = np.ascontiguousarray(x.reshape(B * S, D).astype(np.float32))
    sel, w = _route(xf, Wg, bg, top_k)
    idx_list, gate_list = [], []
    for e in range(E):
        hit = (sel == e)                    # [T, K]
        tok = np.nonzero(hit.any(axis=1))[0]
        kslot = hit[tok].argmax(axis=1)
        idx_list.append(tok)
        gate_list.append(w[tok, kslot])
    C = max(128, int(-(-max(len(t) for t in idx_list) // 64)) * 64)
    return xf, idx_list, gate_list, C


def _pack_inputs(xf, idx_list, C, W1, b1, W2, b2):
    xf_bf = xf.astype(ml_dtypes.bfloat16)
    in_maps = []
    for e in range(E):
        tok = idx_list[e]
        xe = np.zeros((C, D), dtype=ml_dtypes.bfloat16)
        xe[: len(tok)] = xf_bf[tok]
        in_maps.append(
            {
                # [128 d_sub, KD, C] -> flat [128, KD*C]
                "xT": np.ascontiguousarray(
                    xe.reshape(C, KD, 128).transpose(2, 1, 0).reshape(128, KD * C)
                ),
                # [128 d_sub, KF, KD, 128 f] -> flat [128, KF*KD*128]
                "w1": np.ascontiguousarray(
                    W1[e].astype(ml_dtypes.bfloat16)
                    .reshape(KD, 128, KF, 128).transpose(1, 2, 0, 3)
                    .reshape(128, KF * KD * 128)
                ),
                # [128 f_sub, KD, KF, 128 d] -> flat [128, KD*KF*128]
                "w2": np.ascontiguousarray(
                    W2[e].astype(ml_dtypes.bfloat16)
                    .reshape(KF, 128, KD, 128).transpose(1, 2, 0, 3)
                    .reshape(128, KD * KF * 128)
                ),
                "b1": np.ascontiguousarray(b1[e].reshape(KF, 128).T.astype(np.float32)),
                "b2": np.ascontiguousarray(b2[e].reshape(KD, 128).T.astype(np.float32)),
            }
        )
    return in_maps


def _combine(results, idx_list, gate_list, C, T):
    out = np.zeros((T, D), dtype=np.float32)
    for e in range(E):
        tok = idx_list[e]
        if len(tok) == 0:
            continue
        y_pack = results[e]["yT"]                           # [128, KD*C] bf16
        ye = (
            y_pack.reshape(128, KD, C).transpose(2, 1, 0).reshape(C, D)[: len(tok)]
            .astype(np.float32)
        )
        out[tok] += gate_list[e][:, None] * ye
    return out


def kernel(x, W1, b1, W2, b2, Wg, bg, top_k):
    x = np.asarray(x)
    W1 = np.asarray(W1, dtype=np.float32)
    b1 = np.asarray(b1, dtype=np.float32)
    W2 = np.asarray(W2, dtype=np.float32)
    b2 = np.asarray(b2, dtype=np.float32)
    Wg = np.asarray(Wg, dtype=np.float32)
    bg = np.asarray(bg, dtype=np.float32)
    top_k = int(np.asarray(top_k))

    B, S, Din = x.shape
    xf, idx_list, gate_list, C = _plan(x, Wg, bg, top_k)
    nc = _get_kernel(C)
    in_maps = _pack_inputs(xf, idx_list, C, W1, b1, W2, b2)
    res = run_bass_kernel_spmd(nc, in_maps, list(range(E)))
    out = _combine(res.results, idx_list, gate_list, C, B * S)
    return out.reshape(B, S, Din).astype(np.float32)


# revision 9
# speedup vs baseline: 1.0289x; 1.0289x over previous
"""MoE layer (top-2 routing, 8 experts) on 8 Trainium2 NeuronCores.

Strategy (expert parallelism, per sharding hint):
  - Host computes the gate (logits -> top-k -> softmax) and routes tokens:
    expert e's tokens are gathered, padded to a common capacity C, and sent
    to core e (host-side all-to-all).
  - Core e runs the expert FFN for its tokens in a transpose-free layout:
        mm1:  h^T[f,c] = sum_k W1_blk[k,f].T @ x^T[k,c]
        gelu: ACT engine, exact (erf) Gelu, bias b1 fused
        mm2:  y^T[d,c] = sum_f W2_blk[f,d].T @ h^T[f,c]
    Activations/weights bf16 (full PE rate), accumulation fp32 in PSUM.
  - W1+W2 (16MB bf16) are loaded into SBUF ONCE and stay resident; x is
    preloaded in full.  After the initial ~20MB of DMA the PE never waits
    on HBM, which removes the per-chunk weight-reload stalls of the
    streaming version (88MB HBM traffic -> 21MB).
  - y is stored as bf16 (host combine is fp32) to halve store traffic.

Hardcoded problem shape: x [4, 2048, 1024], E=8 experts, D=1024, F=4096.
"""

import numpy as np
import ml_dtypes

import concourse.bass as bass
import concourse.mybir as mybir
import concourse.tile as tile
from concourse import bacc
from concourse.bass_utils import run_bass_kernel_spmd

D = 1024
F = 4096
E = 8
KD = D // 128   # 8 k-tiles over D
KF = F // 128   # 32 k-tiles over F
NT = 512        # max token chunk (PSUM bank = 512 fp32)

_KERNEL_CACHE = {}


def _chunks(C):
    """Token chunks: one narrow head chunk, then equal wide chunks (<= NT).

    The head chunk (~320) shrinks the x-prefetch the PE needs before its
    first matmul can retire (the whole chunk's x is consumed within the
    first f-tile).  All widths are >= ~264 so LDWEIGHTS (~107ns) stays
    hidden under the matmul streaming time, and multiples of 8.
    """
    head = 320
    if C <= head + 264:
        head = 0 if C <= NT else C // 2 & ~7
    rest = C - head
    nch = -(-rest // NT)
    base = (rest // nch) & ~7
    rem = rest - base * nch
    assert rem % 8 == 0
    widths = ([head] if head else []) + [
        base + 8 * (i < rem // 8) for i in range(nch)
    ]
    out, c0 = [], 0
    for w in widths:
        out.append((c0, w))
        c0 += w
    assert c0 == C
    return out


def _build_kernel(C: int):
    """Per-core expert-FFN kernel for capacity C (multiple of 64)."""
    assert C % 64 == 0
    bf16 = mybir.dt.bfloat16
    f32 = mybir.dt.float32

    nc = bacc.Bacc("TRN2", target_bir_lowering=False, debug=False, num_devices=8)

    # All operands packed host-side into partition-major [128, ...] layouts
    # so every load is one large fully-contiguous-per-partition DMA.
    xT = nc.dram_tensor("xT", [128, KD * C], bf16, kind="ExternalInput")
    w1 = nc.dram_tensor("w1", [128, KF * KD * 128], bf16, kind="ExternalInput")
    w2 = nc.dram_tensor("w2", [128, KD * KF * 128], bf16, kind="ExternalInput")
    b1 = nc.dram_tensor("b1", [128, KF], f32, kind="ExternalInput")
    b2 = nc.dram_tensor("b2", [128, KD], f32, kind="ExternalInput")
    yT = nc.dram_tensor("yT", [128, KD * C], bf16, kind="ExternalOutput")

    W1BLK = 4  # f-tiles per w1 load (1MB DMAs)

    with tile.TileContext(nc) as tc:
        with (
            tc.tile_pool(name="const", bufs=1) as const,
            tc.tile_pool(name="hp", bufs=1) as hp,
            tc.tile_pool(name="yp", bufs=4) as yp,
            tc.tile_pool(name="psA", bufs=4, space="PSUM") as psA,
            tc.tile_pool(name="psB", bufs=4, space="PSUM") as psB,
        ):
            chunks = _chunks(C)

            x_sb = const.tile([128, KD * C], bf16)

            def load_x(chunk, queue):
                c0, w = chunk
                for k in range(KD):
                    queue.dma_start(
                        x_sb[:, k * C + c0 : k * C + c0 + w],
                        xT[:, k * C + c0 : k * C + c0 + w],
                    )

            # Head choreography.  Every dma_start costs ~0.65us of trigger
            # time on its queue engine, so the Sync queue leads with w1
            # immediately (the PE's long-pole dependency) while x for the
            # first two chunks rides the GpSimd queue in parallel.  x for
            # chunks 2+ isn't needed until ~80us+, so it queues on Sync
            # BEHIND the weights instead of stealing HBM bandwidth from w1.
            load_x(chunks[0], nc.gpsimd)
            if len(chunks) > 1:
                load_x(chunks[1], nc.gpsimd)

            # w1: first 4 f-tiles individually (PE consumes an f-tile every
            # ~1.1us warm), then 1MB 4-f blocks; w2 per-d behind them.
            w1_sb = const.tile([128, KF * KD * 128], bf16)
            nc.sync.dma_start(w1_sb[:, 0 : KD * 128], w1[:, 0 : KD * 128])
            b1_sb = const.tile([128, KF], f32)
            nc.sync.dma_start(b1_sb[:], b1[:])
            for f0 in range(1, W1BLK):
                lo, hi = f0 * KD * 128, (f0 + 1) * KD * 128
                nc.sync.dma_start(w1_sb[:, lo:hi], w1[:, lo:hi])
            for f0 in range(W1BLK, KF, W1BLK):
                lo, hi = f0 * KD * 128, (f0 + W1BLK) * KD * 128
                nc.sync.dma_start(w1_sb[:, lo:hi], w1[:, lo:hi])
            b2_sb = const.tile([128, KD], f32)
            nc.sync.dma_start(b2_sb[:], b2[:])
            w2_sb = const.tile([128, KD * KF * 128], bf16)
            for d in range(KD):
                lo, hi = d * KF * 128, (d + 1) * KF * 128
                nc.sync.dma_start(w2_sb[:, lo:hi], w2[:, lo:hi])
            for chunk in chunks[2:]:
                load_x(chunk, nc.sync)

            # HAM warm-up: ~8 junk matmuls on a memset tile keep the PE
            # busy during the DMA head so the clock gate reaches K=8/8
            # (2.4 GHz) before the real work starts.
            warm = const.tile([128, 384], bf16)
            nc.vector.memset(warm[:], 0)
            ps = psA.tile([128, NT], f32)
            for i in range(8):
                nc.tensor.matmul(
                    ps[:, :256], warm[:, :128], warm[:, 128:384],
                    start=(i == 0), stop=(i == 7),
                )
            nc.vector.tensor_copy(warm[:, :256], ps[:, :256])

            for c0, w in chunks:
                h_t = hp.tile([128, KF, NT], bf16)
                for f in range(KF):
                    ps = psA.tile([128, NT], f32)
                    for k in range(KD):
                        nc.tensor.matmul(
                            ps[:, :w],
                            w1_sb[:, (f * KD + k) * 128 : (f * KD + k + 1) * 128],
                            x_sb[:, k * C + c0 : k * C + c0 + w],
                            start=(k == 0),
                            stop=(k == KD - 1),
                        )
                    nc.scalar.activation(
                        h_t[:, f, :w],
                        ps[:, :w],
                        mybir.ActivationFunctionType.Gelu,
                        bias=b1_sb[:, f : f + 1],
                    )

                for d in range(KD):
                    ps2 = psB.tile([128, NT], f32)
                    for k2 in range(KF):
                        nc.tensor.matmul(
                            ps2[:, :w],
                            w2_sb[:, (d * KF + k2) * 128 : (d * KF + k2 + 1) * 128],
                            h_t[:, k2, :w],
                            start=(k2 == 0),
                            stop=(k2 == KF - 1),
                        )
                    y_t = yp.tile([128, NT], bf16)
                    nc.vector.tensor_scalar_add(y_t[:, :w], ps2[:, :w], b2_sb[:, d : d + 1])
                    # y stores on the Sync (HWDGE) queue: the end-of-kernel
                    # drain of the GpSimd SWDGE queue costs ~5us extra.
                    nc.sync.dma_start(yT[:, d * C + c0 : d * C + c0 + w], y_t[:, :w])

    nc.compile()
    return nc


def _get_kernel(C: int):
    if C not in _KERNEL_CACHE:
        _KERNEL_CACHE[C] = _build_kernel(C)
    return _KERNEL_CACHE[C]


def _route(xf, Wg, bg, top_k):
    """Replicate the reference gate: logits -> top-k -> softmax."""
    logits = xf.astype(np.float32) @ Wg.astype(np.float32) + bg.astype(np.float32)
    # jax.lax.top_k: values sorted descending, ties broken by lower index.
    order = np.argsort(-logits, axis=1, kind="stable")
    sel = order[:, :top_k]                                      # [T, K]
    vals = np.take_along_axis(logits, sel, axis=1)              # [T, K]
    vmax = vals.max(axis=1, keepdims=True)
    ex = np.exp((vals - vmax).astype(np.float32))
    w = ex / ex.sum(axis=1, keepdims=True)                      # [T, K]
    return sel, w.astype(np.float32)


def _plan(x, Wg, bg, top_k):
    """Routing plan: token indices + gate weight per expert, capacity C."""
    B, S, _ = x.shape
    xf = np.ascontiguousarray(x.reshape(B * S, D).astype(np.float32))
    sel, w = _route(xf, Wg, bg, top_k)
    idx_list, gate_list = [], []
    for e in range(E):
        hit = (sel == e)                    # [T, K]
        tok = np.nonzero(hit.any(axis=1))[0]
        kslot = hit[tok].argmax(axis=1)
        idx_list.append(tok)
        gate_list.append(w[tok, kslot])
    C = max(128, int(-(-max(len(t) for t in idx_list) // 64)) * 64)
    return xf, idx_list, gate_list, C


def _pack_inputs(xf, idx_list, C, W1, b1, W2, b2):
    xf_bf = xf.astype(ml_dtypes.bfloat16)
    in_maps = []
    for e in range(E):
        tok = idx_list[e]
        xe = np.zeros((C, D), dtype=ml_dtypes.bfloat16)
        xe[: len(tok)] = xf_bf[tok]
        in_maps.append(
            {
                # [128 d_sub, KD, C] -> flat [128, KD*C]
                "xT": np.ascontiguousarray(
                    xe.reshape(C, KD, 128).transpose(2, 1, 0).reshape(128, KD * C)
                ),
                # [128 d_sub, KF, KD, 128 f] -> flat [128, KF*KD*128]
                "w1": np.ascontiguousarray(
                    W1[e].astype(ml_dtypes.bfloat16)
                    .reshape(KD, 128, KF, 128).transpose(1, 2, 0, 3)
                    .reshape(128, KF * KD * 128)
                ),
                # [128 f_sub, KD, KF, 128 d] -> flat [128, KD*KF*128]
                "w2": np.ascontiguousarray(
                    W2[e].astype(ml_dtypes.bfloat16)
                    .reshape(KF, 128, KD, 128).transpose(1, 2, 0, 3)
                    .reshape(128, KD * KF * 128)
                ),
                "b1": np.ascontiguousarray(b1[e].reshape(KF, 128).T.astype(np.float32)),
                "b2": np.ascontiguousarray(b2[e].reshape(KD, 128).T.astype(np.float32)),
            }
        )
    return in_maps


def _combine(results, idx_list, gate_list, C, T):
    out = np.zeros((T, D), dtype=np.float32)
    for e in range(E):
        tok = idx_list[e]
        if len(tok) == 0:
            continue
        y_pack = results[e]["yT"]                           # [128, KD*C] bf16
        ye = (
            y_pack.reshape(128, KD, C).transpose(2, 1, 0).reshape(C, D)[: len(tok)]
            .astype(np.float32)
        )
        out[tok] += gate_list[e][:, None] * ye
    return out


def kernel(x, W1, b1, W2, b2, Wg, bg, top_k):
    x = np.asarray(x)
    W1 = np.asarray(W1, dtype=np.float32)
    b1 = np.asarray(b1, dtype=np.float32)
    W2 = np.asarray(W2, dtype=np.float32)
    b2 = np.asarray(b2, dtype=np.float32)
    Wg = np.asarray(Wg, dtype=np.float32)
    bg = np.asarray(bg, dtype=np.float32)
    top_k = int(np.asarray(top_k))

    B, S, Din = x.shape
    xf, idx_list, gate_list, C = _plan(x, Wg, bg, top_k)
    nc = _get_kernel(C)
    in_maps = _pack_inputs(xf, idx_list, C, W1, b1, W2, b2)
    res = run_bass_kernel_spmd(nc, in_maps, list(range(E)))
    out = _combine(res.results, idx_list, gate_list, C, B * S)
    return out.reshape(B, S, Din).astype(np.float32)


# revision 16
# speedup vs baseline: 1.0641x; 1.0342x over previous
"""MoE layer (top-2 routing, 8 experts) on 8 Trainium2 NeuronCores.

Strategy (expert parallelism, per sharding hint):
  - Host computes the gate (logits -> top-k -> softmax) and routes tokens:
    expert e's tokens are gathered, padded to a common capacity C, and sent
    to core e (host-side all-to-all).
  - Core e runs the expert FFN for its tokens in a transpose-free layout:
        mm1:  h^T[f,c] = sum_k W1_blk[k,f].T @ x^T[k,c]
        gelu: ACT engine, exact (erf) Gelu, bias b1 fused
        mm2:  y^T[d,c] = sum_f W2_blk[f,d].T @ h^T[f,c]
    Activations/weights bf16 (full PE rate), accumulation fp32 in PSUM.
  - W1+W2 (16MB bf16) are loaded into SBUF ONCE and stay resident; x is
    preloaded in full.  After the initial ~20MB of DMA the PE never waits
    on HBM, which removes the per-chunk weight-reload stalls of the
    streaming version (88MB HBM traffic -> 21MB).
  - y is stored as bf16 (host combine is fp32) to halve store traffic.

Hardcoded problem shape: x [4, 2048, 1024], E=8 experts, D=1024, F=4096.
"""

import numpy as np
import ml_dtypes

import concourse.bass as bass
import concourse.mybir as mybir
import concourse.tile as tile
from concourse import bacc
from concourse.bass_utils import run_bass_kernel_spmd

D = 1024
F = 4096
E = 8
KD = D // 128   # 8 k-tiles over D
KF = F // 128   # 32 k-tiles over F
NT = 512        # max token chunk (PSUM bank = 512 fp32)

_KERNEL_CACHE = {}


def _chunks(C):
    """Token chunks: one narrow head chunk, then equal wide chunks (<= NT).

    The head chunk (~320) shrinks the x-prefetch the PE needs before its
    first matmul can retire (the whole chunk's x is consumed within the
    first f-tile).  All widths are >= ~264 so LDWEIGHTS (~107ns) stays
    hidden under the matmul streaming time, and multiples of 8.
    """
    head = 320
    if C <= head + 264:
        head = 0 if C <= NT else C // 2 & ~7
    rest = C - head
    nch = -(-rest // NT)
    base = (rest // nch) & ~7
    rem = rest - base * nch
    assert rem % 8 == 0
    widths = ([head] if head else []) + [
        base + 8 * (i < rem // 8) for i in range(nch)
    ]
    out, c0 = [], 0
    for w in widths:
        out.append((c0, w))
        c0 += w
    assert c0 == C
    return out


def _build_kernel(C: int):
    """Per-core expert-FFN kernel for capacity C (multiple of 64)."""
    assert C % 64 == 0
    bf16 = mybir.dt.bfloat16
    f32 = mybir.dt.float32

    nc = bacc.Bacc("TRN2", target_bir_lowering=False, debug=False, num_devices=8)

    # All operands packed host-side into partition-major [128, ...] layouts
    # so every load is one large fully-contiguous-per-partition DMA.
    xT = nc.dram_tensor("xT", [128, KD, C], bf16, kind="ExternalInput")
    w1 = nc.dram_tensor("w1", [128, KF * KD * 128], bf16, kind="ExternalInput")
    w2 = nc.dram_tensor("w2", [128, KD * KF * 128], bf16, kind="ExternalInput")
    b1 = nc.dram_tensor("b1", [128, KF], f32, kind="ExternalInput")
    b2 = nc.dram_tensor("b2", [128, KD], f32, kind="ExternalInput")
    yT = nc.dram_tensor("yT", [128, KD * C], bf16, kind="ExternalOutput")

    W1BLK = 4  # f-tiles per w1 load (1MB DMAs)

    with tile.TileContext(nc) as tc:
        with (
            tc.tile_pool(name="const", bufs=1) as const,
            tc.tile_pool(name="hp", bufs=1) as hp,
            tc.tile_pool(name="yp", bufs=4) as yp,
            tc.tile_pool(name="psA", bufs=4, space="PSUM") as psA,
            tc.tile_pool(name="psB", bufs=4, space="PSUM") as psB,
        ):
            chunks = _chunks(C)

            x_sb = const.tile([128, KD, C], bf16)

            def load_x(chunk, queue):
                c0, w = chunk
                queue.dma_start(x_sb[:, :, c0 : c0 + w], xT[:, :, c0 : c0 + w])

            # Head choreography.  Every dma_start costs ~0.65us of trigger
            # time on its queue engine, so the Sync queue leads with w1
            # immediately (the PE's long-pole dependency) while x for the
            # first two chunks rides the GpSimd queue in parallel (one
            # batched trigger per chunk -- per-k triggers arrive 0.65us
            # apart and stutter the PE's first f-tile, keeping the HAM
            # clock gate cold).  x for chunks 2+ isn't needed until ~80us+,
            # so it queues on Sync BEHIND the weights instead of stealing
            # HBM bandwidth from w1.
            c00, w0 = chunks[0]
            nc.gpsimd.dma_start(x_sb[:, 0:4, c00 : c00 + w0], xT[:, 0:4, c00 : c00 + w0])
            nc.gpsimd.dma_start(x_sb[:, 4:8, c00 : c00 + w0], xT[:, 4:8, c00 : c00 + w0])
            if len(chunks) > 1:
                load_x(chunks[1], nc.gpsimd)

            # w1: first 4 f-tiles individually (PE consumes an f-tile every
            # ~1.1us warm), then 1MB 4-f blocks; w2 per-d behind them.
            w1_sb = const.tile([128, KF * KD * 128], bf16)
            nc.sync.dma_start(w1_sb[:, 0 : KD * 128], w1[:, 0 : KD * 128])
            b1_sb = const.tile([128, KF], f32)
            nc.sync.dma_start(b1_sb[:], b1[:])
            for f0 in range(1, W1BLK):
                lo, hi = f0 * KD * 128, (f0 + 1) * KD * 128
                nc.sync.dma_start(w1_sb[:, lo:hi], w1[:, lo:hi])
            for f0 in range(W1BLK, KF, W1BLK):
                lo, hi = f0 * KD * 128, (f0 + W1BLK) * KD * 128
                nc.sync.dma_start(w1_sb[:, lo:hi], w1[:, lo:hi])
            b2_sb = const.tile([128, KD], f32)
            nc.sync.dma_start(b2_sb[:], b2[:])
            w2_sb = const.tile([128, KD * KF * 128], bf16)
            for d in range(KD):
                lo, hi = d * KF * 128, (d + 1) * KF * 128
                nc.sync.dma_start(w2_sb[:, lo:hi], w2[:, lo:hi])
            for chunk in chunks[2:]:
                load_x(chunk, nc.sync)

            # HAM warm-up: ~8 junk matmuls on a memset tile keep the PE
            # busy during the DMA head so the clock gate reaches K=8/8
            # (2.4 GHz) before the real work starts.
            warm = const.tile([128, 384], bf16)
            nc.vector.memset(warm[:], 0)
            ps = psA.tile([128, NT], f32)
            for i in range(8):
                nc.tensor.matmul(
                    ps[:, :256], warm[:, :128], warm[:, 128:384],
                    start=(i == 0), stop=(i == 7),
                )
            nc.vector.tensor_copy(warm[:, :256], ps[:, :256])

            for c0, w in chunks:
                h_t = hp.tile([128, KF, NT], bf16)
                for f in range(KF):
                    ps = psA.tile([128, NT], f32)
                    for k in range(KD):
                        nc.tensor.matmul(
                            ps[:, :w],
                            w1_sb[:, (f * KD + k) * 128 : (f * KD + k + 1) * 128],
                            x_sb[:, k, c0 : c0 + w],
                            start=(k == 0),
                            stop=(k == KD - 1),
                        )
                    nc.scalar.activation(
                        h_t[:, f, :w],
                        ps[:, :w],
                        mybir.ActivationFunctionType.Gelu,
                        bias=b1_sb[:, f : f + 1],
                    )

                for d in range(KD):
                    ps2 = psB.tile([128, NT], f32)
                    for k2 in range(KF):
                        nc.tensor.matmul(
                            ps2[:, :w],
                            w2_sb[:, (d * KF + k2) * 128 : (d * KF + k2 + 1) * 128],
                            h_t[:, k2, :w],
                            start=(k2 == 0),
                            stop=(k2 == KF - 1),
                        )
                    y_t = yp.tile([128, NT], bf16)
                    nc.vector.tensor_scalar_add(y_t[:, :w], ps2[:, :w], b2_sb[:, d : d + 1])
                    # y stores on the Sync (HWDGE) queue: the end-of-kernel
                    # drain of the GpSimd SWDGE queue costs ~5us extra.
                    nc.sync.dma_start(yT[:, d * C + c0 : d * C + c0 + w], y_t[:, :w])

    nc.compile()
    return nc


def _get_kernel(C: int):
    if C not in _KERNEL_CACHE:
        _KERNEL_CACHE[C] = _build_kernel(C)
    return _KERNEL_CACHE[C]


def _route(xf, Wg, bg, top_k):
    """Replicate the reference gate: logits -> top-k -> softmax."""
    logits = xf.astype(np.float32) @ Wg.astype(np.float32) + bg.astype(np.float32)
    # jax.lax.top_k: values sorted descending, ties broken by lower index.
    order = np.argsort(-logits, axis=1, kind="stable")
    sel = order[:, :top_k]                                      # [T, K]
    vals = np.take_along_axis(logits, sel, axis=1)              # [T, K]
    vmax = vals.max(axis=1, keepdims=True)
    ex = np.exp((vals - vmax).astype(np.float32))
    w = ex / ex.sum(axis=1, keepdims=True)                      # [T, K]
    return sel, w.astype(np.float32)


def _plan(x, Wg, bg, top_k):
    """Routing plan: token indices + gate weight per expert, capacity C.

    Capacity is capped at the mean load (T*top_k/E rounded to 64): tokens
    beyond an expert's capacity (~1% of pairs for balanced random routing)
    are returned as overflow and combined on the host in fp32.  This keeps
    every core's (identical) kernel at the balanced-load PE floor instead
    of the max-loaded expert's.
    """
    B, S, _ = x.shape
    T = B * S
    xf = np.ascontiguousarray(x.reshape(T, D).astype(np.float32))
    sel, w = _route(xf, Wg, bg, top_k)
    idx_list, gate_list = [], []
    for e in range(E):
        hit = (sel == e)                    # [T, K]
        tok = np.nonzero(hit.any(axis=1))[0]
        kslot = hit[tok].argmax(axis=1)
        idx_list.append(tok)
        gate_list.append(w[tok, kslot])
    cap = -(-(T * top_k // E) // 64) * 64
    need = max(len(t) for t in idx_list)
    # Keep host-side overflow bounded (<5% of pairs) for skewed routings.
    while sum(max(0, len(t) - cap) for t in idx_list) > 0.05 * T * top_k:
        cap += 64
    C = max(128, min(int(-(-need // 64)) * 64, cap))
    overflow = [(idx_list[e][C:], gate_list[e][C:]) for e in range(E)]
    idx_list = [t[:C] for t in idx_list]
    gate_list = [g[:C] for g in gate_list]
    return xf, idx_list, gate_list, C, overflow


def _pack_inputs(xf, idx_list, C, W1, b1, W2, b2):
    xf_bf = xf.astype(ml_dtypes.bfloat16)
    in_maps = []
    for e in range(E):
        tok = idx_list[e]
        xe = np.zeros((C, D), dtype=ml_dtypes.bfloat16)
        xe[: len(tok)] = xf_bf[tok]
        in_maps.append(
            {
                # [128 d_sub, KD, C]
                "xT": np.ascontiguousarray(xe.reshape(C, KD, 128).transpose(2, 1, 0)),
                # [128 d_sub, KF, KD, 128 f] -> flat [128, KF*KD*128]
                "w1": np.ascontiguousarray(
                    W1[e].astype(ml_dtypes.bfloat16)
                    .reshape(KD, 128, KF, 128).transpose(1, 2, 0, 3)
                    .reshape(128, KF * KD * 128)
                ),
                # [128 f_sub, KD, KF, 128 d] -> flat [128, KD*KF*128]
                "w2": np.ascontiguousarray(
                    W2[e].astype(ml_dtypes.bfloat16)
                    .reshape(KF, 128, KD, 128).transpose(1, 2, 0, 3)
                    .reshape(128, KD * KF * 128)
                ),
                "b1": np.ascontiguousarray(b1[e].reshape(KF, 128).T.astype(np.float32)),
                "b2": np.ascontiguousarray(b2[e].reshape(KD, 128).T.astype(np.float32)),
            }
        )
    return in_maps


def _erf(v):
    """Vectorized erf, Abramowitz-Stegun 7.1.26 (|err| < 1.5e-7)."""
    s = np.sign(v)
    v = np.abs(v)
    t = 1.0 / (1.0 + 0.3275911 * v)
    poly = t * (
        0.254829592
        + t * (-0.284496736 + t * (1.421413741 + t * (-1.453152027 + t * 1.061405429)))
    )
    return s * (1.0 - poly * np.exp(-v * v))


def _combine(results, idx_list, gate_list, C, T, overflow, xf, W1, b1, W2, b2):
    out = np.zeros((T, D), dtype=np.float32)
    for e in range(E):
        tok = idx_list[e]
        if len(tok) == 0:
            continue
        y_pack = results[e]["yT"]                           # [128, KD*C] bf16
        ye = (
            y_pack.reshape(128, KD, C).transpose(2, 1, 0).reshape(C, D)[: len(tok)]
            .astype(np.float32)
        )
        out[tok] += gate_list[e][:, None] * ye
    # Overflow pairs (beyond capacity) in fp32 on the host.
    for e in range(E):
        tok, g = overflow[e]
        if len(tok) == 0:
            continue
        u = xf[tok] @ W1[e] + b1[e]
        h = u * 0.5 * (1.0 + _erf(u / np.sqrt(2.0)))
        ye = h @ W2[e] + b2[e]
        out[tok] += g[:, None] * ye
    return out


def kernel(x, W1, b1, W2, b2, Wg, bg, top_k):
    x = np.asarray(x)
    W1 = np.asarray(W1, dtype=np.float32)
    b1 = np.asarray(b1, dtype=np.float32)
    W2 = np.asarray(W2, dtype=np.float32)
    b2 = np.asarray(b2, dtype=np.float32)
    Wg = np.asarray(Wg, dtype=np.float32)
    bg = np.asarray(bg, dtype=np.float32)
    top_k = int(np.asarray(top_k))

    B, S, Din = x.shape
    xf, idx_list, gate_list, C, overflow = _plan(x, Wg, bg, top_k)
    nc = _get_kernel(C)
    in_maps = _pack_inputs(xf, idx_list, C, W1, b1, W2, b2)
    res = run_bass_kernel_spmd(nc, in_maps, list(range(E)))
    out = _combine(
        res.results, idx_list, gate_list, C, B * S, overflow, xf, W1, b1, W2, b2
    )
    return out.reshape(B, S, Din).astype(np.float32)


# revision 19
# speedup vs baseline: 1.0685x; 1.0041x over previous
"""MoE layer (top-2 routing, 8 experts) on 8 Trainium2 NeuronCores.

Strategy (expert parallelism, per sharding hint):
  - Host computes the gate (logits -> top-k -> softmax) and routes tokens:
    expert e's tokens are gathered, padded to a common capacity C, and sent
    to core e (host-side all-to-all).
  - Core e runs the expert FFN for its tokens in a transpose-free layout:
        mm1:  h^T[f,c] = sum_k W1_blk[k,f].T @ x^T[k,c]
        gelu: ACT engine, exact (erf) Gelu, bias b1 fused
        mm2:  y^T[d,c] = sum_f W2_blk[f,d].T @ h^T[f,c]
    Activations/weights bf16 (full PE rate), accumulation fp32 in PSUM.
  - W1+W2 (16MB bf16) are loaded into SBUF ONCE and stay resident; x is
    preloaded in full.  After the initial ~20MB of DMA the PE never waits
    on HBM, which removes the per-chunk weight-reload stalls of the
    streaming version (88MB HBM traffic -> 21MB).
  - y is stored as bf16 (host combine is fp32) to halve store traffic.

Hardcoded problem shape: x [4, 2048, 1024], E=8 experts, D=1024, F=4096.
"""

import numpy as np
import ml_dtypes

import concourse.bass as bass
import concourse.mybir as mybir
import concourse.tile as tile
from concourse import bacc
from concourse.bass_utils import run_bass_kernel_spmd

D = 1024
F = 4096
E = 8
KD = D // 128   # 8 k-tiles over D
KF = F // 128   # 32 k-tiles over F
NT = 512        # max token chunk (PSUM bank = 512 fp32)

_KERNEL_CACHE = {}


def _chunks(C):
    """Token chunks: one narrow head chunk, then equal wide chunks (<= NT).

    The head chunk (~320) shrinks the x-prefetch the PE needs before its
    first matmul can retire (the whole chunk's x is consumed within the
    first f-tile).  All widths are >= ~264 so LDWEIGHTS (~107ns) stays
    hidden under the matmul streaming time, and multiples of 8.
    """
    head = 320
    if C <= head + 264:
        head = 0 if C <= NT else C // 2 & ~7
    rest = C - head
    nch = -(-rest // NT)
    base = (rest // nch) & ~7
    rem = rest - base * nch
    assert rem % 8 == 0
    widths = ([head] if head else []) + [
        base + 8 * (i < rem // 8) for i in range(nch)
    ]
    out, c0 = [], 0
    for w in widths:
        out.append((c0, w))
        c0 += w
    assert c0 == C
    return out


def _build_kernel(C: int):
    """Per-core expert-FFN kernel for capacity C (multiple of 64)."""
    assert C % 64 == 0
    bf16 = mybir.dt.bfloat16
    f32 = mybir.dt.float32

    nc = bacc.Bacc("TRN2", target_bir_lowering=False, debug=False, num_devices=8)

    # All operands packed host-side into partition-major [128, ...] layouts
    # so every load is one large fully-contiguous-per-partition DMA.
    xT = nc.dram_tensor("xT", [128, KD, C], bf16, kind="ExternalInput")
    w1 = nc.dram_tensor("w1", [128, KF * KD * 128], bf16, kind="ExternalInput")
    w2 = nc.dram_tensor("w2", [128, KD * KF * 128], bf16, kind="ExternalInput")
    b1 = nc.dram_tensor("b1", [128, KF], f32, kind="ExternalInput")
    b2 = nc.dram_tensor("b2", [128, KD], f32, kind="ExternalInput")
    yT = nc.dram_tensor("yT", [128, KD * C], bf16, kind="ExternalOutput")

    W1BLK = 4  # f-tiles per w1 load (1MB DMAs)

    with tile.TileContext(nc) as tc:
        with (
            tc.tile_pool(name="const", bufs=1) as const,
            tc.tile_pool(name="hp", bufs=1) as hp,
            tc.tile_pool(name="yp", bufs=4) as yp,
            tc.tile_pool(name="psA", bufs=4, space="PSUM") as psA,
            tc.tile_pool(name="psB", bufs=4, space="PSUM") as psB,
        ):
            chunks = _chunks(C)

            x_sb = const.tile([128, KD, C], bf16)

            def load_x(chunk, queue):
                c0, w = chunk
                queue.dma_start(x_sb[:, :, c0 : c0 + w], xT[:, :, c0 : c0 + w])

            # Head choreography.  Every dma_start costs ~0.65us of trigger
            # time on its queue engine, so the Sync queue leads with w1
            # immediately (the PE's long-pole dependency) while x for the
            # first two chunks rides the GpSimd queue in parallel (one
            # batched trigger per chunk -- per-k triggers arrive 0.65us
            # apart and stutter the PE's first f-tile, keeping the HAM
            # clock gate cold).  x for chunks 2+ isn't needed until ~80us+,
            # so it queues on Sync BEHIND the weights instead of stealing
            # HBM bandwidth from w1.
            c00, w0 = chunks[0]
            nc.gpsimd.dma_start(x_sb[:, 0:4, c00 : c00 + w0], xT[:, 0:4, c00 : c00 + w0])
            nc.gpsimd.dma_start(x_sb[:, 4:8, c00 : c00 + w0], xT[:, 4:8, c00 : c00 + w0])

            # w1: first 4 f-tiles individually (PE consumes an f-tile every
            # ~1.1us warm), then 1MB 4-f blocks; w2 per-d behind them.
            w1_sb = const.tile([128, KF * KD * 128], bf16)
            nc.sync.dma_start(w1_sb[:, 0 : KD * 128], w1[:, 0 : KD * 128])
            b1_sb = const.tile([128, KF], f32)
            nc.sync.dma_start(b1_sb[:], b1[:])
            for f0 in range(1, W1BLK):
                lo, hi = f0 * KD * 128, (f0 + 1) * KD * 128
                nc.sync.dma_start(w1_sb[:, lo:hi], w1[:, lo:hi])
            for f0 in range(W1BLK, KF, W1BLK):
                lo, hi = f0 * KD * 128, (f0 + W1BLK) * KD * 128
                nc.sync.dma_start(w1_sb[:, lo:hi], w1[:, lo:hi])
            b2_sb = const.tile([128, KD], f32)
            nc.sync.dma_start(b2_sb[:], b2[:])
            w2_sb = const.tile([128, KD * KF * 128], bf16)
            for d in range(KD):
                lo, hi = d * KF * 128, (d + 1) * KF * 128
                nc.sync.dma_start(w2_sb[:, lo:hi], w2[:, lo:hi])
            # chunk 1's x is first needed when chunk 0's mm2 ends (~77us);
            # behind w2 on Sync it lands ~65us without costing w1 any HBM
            # bandwidth in the critical 8-30us window.
            for chunk in chunks[1:]:
                load_x(chunk, nc.sync)

            # HAM warm-up: ~8 junk matmuls on a memset tile keep the PE
            # busy during the DMA head so the clock gate reaches K=8/8
            # (2.4 GHz) before the real work starts.
            warm = const.tile([128, 384], bf16)
            nc.vector.memset(warm[:], 0)
            ps = psA.tile([128, NT], f32)
            for i in range(12):
                nc.tensor.matmul(
                    ps[:, :256], warm[:, :128], warm[:, 128:384],
                    start=(i == 0), stop=(i == 11),
                )
            nc.vector.tensor_copy(warm[:, :256], ps[:, :256])

            for c0, w in chunks:
                h_t = hp.tile([128, KF, NT], bf16)
                for f in range(KF):
                    ps = psA.tile([128, NT], f32)
                    for k in range(KD):
                        nc.tensor.matmul(
                            ps[:, :w],
                            w1_sb[:, (f * KD + k) * 128 : (f * KD + k + 1) * 128],
                            x_sb[:, k, c0 : c0 + w],
                            start=(k == 0),
                            stop=(k == KD - 1),
                        )
                    nc.scalar.activation(
                        h_t[:, f, :w],
                        ps[:, :w],
                        mybir.ActivationFunctionType.Gelu,
                        bias=b1_sb[:, f : f + 1],
                    )

                for d in range(KD):
                    ps2 = psB.tile([128, NT], f32)
                    for k2 in range(KF):
                        nc.tensor.matmul(
                            ps2[:, :w],
                            w2_sb[:, (d * KF + k2) * 128 : (d * KF + k2 + 1) * 128],
                            h_t[:, k2, :w],
                            start=(k2 == 0),
                            stop=(k2 == KF - 1),
                        )
                    y_t = yp.tile([128, NT], bf16)
                    nc.vector.tensor_scalar_add(y_t[:, :w], ps2[:, :w], b2_sb[:, d : d + 1])
                    # y stores on the Sync (HWDGE) queue: the end-of-kernel
                    # drain of the GpSimd SWDGE queue costs ~5us extra.
                    nc.sync.dma_start(yT[:, d * C + c0 : d * C + c0 + w], y_t[:, :w])

    nc.compile()
    return nc


def _get_kernel(C: int):
    if C not in _KERNEL_CACHE:
        _KERNEL_CACHE[C] = _build_kernel(C)
    return _KERNEL_CACHE[C]


def _route(xf, Wg, bg, top_k):
    """Replicate the reference gate: logits -> top-k -> softmax."""
    logits = xf.astype(np.float32) @ Wg.astype(np.float32) + bg.astype(np.float32)
    # jax.lax.top_k: values sorted descending, ties broken by lower index.
    order = np.argsort(-logits, axis=1, kind="stable")
    sel = order[:, :top_k]                                      # [T, K]
    vals = np.take_along_axis(logits, sel, axis=1)              # [T, K]
    vmax = vals.max(axis=1, keepdims=True)
    ex = np.exp((vals - vmax).astype(np.float32))
    w = ex / ex.sum(axis=1, keepdims=True)                      # [T, K]
    return sel, w.astype(np.float32)


def _plan(x, Wg, bg, top_k):
    """Routing plan: token indices + gate weight per expert, capacity C.

    Capacity is capped at the mean load (T*top_k/E rounded to 64): tokens
    beyond an expert's capacity (~1% of pairs for balanced random routing)
    are returned as overflow and combined on the host in fp32.  This keeps
    every core's (identical) kernel at the balanced-load PE floor instead
    of the max-loaded expert's.
    """
    B, S, _ = x.shape
    T = B * S
    xf = np.ascontiguousarray(x.reshape(T, D).astype(np.float32))
    sel, w = _route(xf, Wg, bg, top_k)
    idx_list, gate_list = [], []
    for e in range(E):
        hit = (sel == e)                    # [T, K]
        tok = np.nonzero(hit.any(axis=1))[0]
        kslot = hit[tok].argmax(axis=1)
        idx_list.append(tok)
        gate_list.append(w[tok, kslot])
    cap = -(-(T * top_k // E) // 64) * 64
    need = max(len(t) for t in idx_list)
    # Keep host-side overflow bounded (<5% of pairs) for skewed routings.
    while sum(max(0, len(t) - cap) for t in idx_list) > 0.05 * T * top_k:
        cap += 64
    C = max(128, min(int(-(-need // 64)) * 64, cap))
    overflow = [(idx_list[e][C:], gate_list[e][C:]) for e in range(E)]
    idx_list = [t[:C] for t in idx_list]
    gate_list = [g[:C] for g in gate_list]
    return xf, idx_list, gate_list, C, overflow


def _pack_inputs(xf, idx_list, C, W1, b1, W2, b2):
    xf_bf = xf.astype(ml_dtypes.bfloat16)
    in_maps = []
    for e in range(E):
        tok = idx_list[e]
        xe = np.zeros((C, D), dtype=ml_dtypes.bfloat16)
        xe[: len(tok)] = xf_bf[tok]
        in_maps.append(
            {
                # [128 d_sub, KD, C]
                "xT": np.ascontiguousarray(xe.reshape(C, KD, 128).transpose(2, 1, 0)),
                # [128 d_sub, KF, KD, 128 f] -> flat [128, KF*KD*128]
                "w1": np.ascontiguousarray(
                    W1[e].astype(ml_dtypes.bfloat16)
                    .reshape(KD, 128, KF, 128).transpose(1, 2, 0, 3)
                    .reshape(128, KF * KD * 128)
                ),
                # [128 f_sub, KD, KF, 128 d] -> flat [128, KD*KF*128]
                "w2": np.ascontiguousarray(
                    W2[e].astype(ml_dtypes.bfloat16)
                    .reshape(KF, 128, KD, 128).transpose(1, 2, 0, 3)
                    .reshape(128, KD * KF * 128)
                ),
                "b1": np.ascontiguousarray(b1[e].reshape(KF, 128).T.astype(np.float32)),
                "b2": np.ascontiguousarray(b2[e].reshape(KD, 128).T.astype(np.float32)),
            }
        )
    return in_maps


def _erf(v):
    """Vectorized erf, Abramowitz-Stegun 7.1.26 (|err| < 1.5e-7)."""
    s = np.sign(v)
    v = np.abs(v)
    t = 1.0 / (1.0 + 0.3275911 * v)
    poly = t * (
        0.254829592
        + t * (-0.284496736 + t * (1.421413741 + t * (-1.453152027 + t * 1.061405429)))
    )
    return s * (1.0 - poly * np.exp(-v * v))


def _combine(results, idx_list, gate_list, C, T, overflow, xf, W1, b1, W2, b2):
    out = np.zeros((T, D), dtype=np.float32)
    for e in range(E):
        tok = idx_list[e]
        if len(tok) == 0:
            continue
        y_pack = results[e]["yT"]                           # [128, KD*C] bf16
        ye = (
            y_pack.reshape(128, KD, C).transpose(2, 1, 0).reshape(C, D)[: len(tok)]
            .astype(np.float32)
        )
        out[tok] += gate_list[e][:, None] * ye
    # Overflow pairs (beyond capacity) in fp32 on the host.
    for e in range(E):
        tok, g = overflow[e]
        if len(tok) == 0:
            continue
        u = xf[tok] @ W1[e] + b1[e]
        h = u * 0.5 * (1.0 + _erf(u / np.sqrt(2.0)))
        ye = h @ W2[e] + b2[e]
        out[tok] += g[:, None] * ye
    return out


def kernel(x, W1, b1, W2, b2, Wg, bg, top_k):
    x = np.asarray(x)
    W1 = np.asarray(W1, dtype=np.float32)
    b1 = np.asarray(b1, dtype=np.float32)
    W2 = np.asarray(W2, dtype=np.float32)
    b2 = np.asarray(b2, dtype=np.float32)
    Wg = np.asarray(Wg, dtype=np.float32)
    bg = np.asarray(bg, dtype=np.float32)
    top_k = int(np.asarray(top_k))

    B, S, Din = x.shape
    xf, idx_list, gate_list, C, overflow = _plan(x, Wg, bg, top_k)
    nc = _get_kernel(C)
    in_maps = _pack_inputs(xf, idx_list, C, W1, b1, W2, b2)
    res = run_bass_kernel_spmd(nc, in_maps, list(range(E)))
    out = _combine(
        res.results, idx_list, gate_list, C, B * S, overflow, xf, W1, b1, W2, b2
    )
    return out.reshape(B, S, Din).astype(np.float32)
